# revision 33
# baseline (speedup 1.0000x reference)
"""Distance-correlation (DcorLoss) kernel for 8 trn2 NeuronCores.

Math: for x, y [n=8192, d=128]:
  a = pairwise_dist(x), b = pairwise_dist(y)   (n x n, symmetric, zero diag)
  A = double_center(a), B = double_center(b)
  dcor = -sqrt(sum(A*B)) / sqrt(sqrt(sum(A*A)) * sqrt(sum(B*B)))

Identities (never materialize A/B):
  sum(HaH o HbH) = sum(at o bt) - 2/n * dot(rs_at, rs_bt) + sum(at)*sum(bt)/n^2
with at = a - mu. sum(a-mu)^2 via the closed form for sum a^2 = sum sq.
Only sum (a-mu)*b and the row sums of a/b need streaming the matrices.

Symmetric block coverage: core c owns row block c (1024 rows). Each unordered
block pair {r, j} is computed once: core c runs 5 column-window "slots"
s=0..4 over windows (c+s) mod 8. Slot 0 = diagonal block, slots 1-3 pairs
counted twice on host, slot 4 pair computed by both ends (counted once each).
Row sums for the mirrored (uncomputed) windows of block c come from COLUMN
sums of slots 1-3 tiles of cores (c+5..c+7) mod 8, computed on-device with
ones^T matmuls accumulated in PSUM across the 8 row chunks of a slot.

Per (128-row x 1024-col) tile pair, the device computes:
  PE:   psum = -2*x_blk^T x  (bf16 K=128) + column norms (bf16 hi/lo K=2)
        + mu^2*I on the slot-0 diagonal sub-block (fp8 DoubleRow identity)
  ACT:  t = sqrt(psum + n_i)  [fp32 row-norm bias], float32r out; the a-side
        carries accum_out -> row sums (consistent: all consumers see the
        same ~fp32 values, which keeps the closed-form sum-of-squares and
        the streamed sums in agreement -- bf16 outputs here cost 2e-2 rel)
  DVE:  row-sum reduce of t_b, (t_a - mu) * t_b accum (stt)
  PE:   ones^T t_a / t_b column sums (slots 1-3, f32r, dst partition 0),
        deferred one iteration so they never block the next psum fill
Cross-core combining is fp64 on host (partials are tiny).
"""

import os

import numpy as np
import ml_dtypes

import concourse.bass as bass
import concourse.tile as tile
from concourse import bacc, mybir
from concourse.bass_utils import run_bass_kernel_spmd

P = 128            # partitions / d
N = 8192           # points
NCORES = 8
BLK = N // NCORES  # 1024 rows per core
CI_N = BLK // P    # 8 row chunks per core
W = 1024           # column window
NSLOT = 5          # symmetric coverage slots
MU = 16.0
F8 = ml_dtypes.float8_e4m3
BF = ml_dtypes.bfloat16

_programs = {}


def _emit_cs(nc, cst, ones1, pend, ci_n):
    aT, bT, ci = pend
    for r, (ssrc, h) in enumerate(((aT, 0), (aT, 1), (bT, 0), (bT, 1))):
        nc.tensor.matmul(
            cst[0:1, r * 512:(r + 1) * 512], ones1[:],
            ssrc[:, bass.ts(h, 512)],
            start=(ci == 0), stop=(ci == ci_n - 1),
            skip_group_check=True, tile_position=(0, 0))


def _build():
    dt = mybir.dt
    f32 = dt.float32
    f32r = dt.float32r
    f8 = dt.float8e4
    bf = dt.bfloat16
    A = mybir.AluOpType
    AF = mybir.ActivationFunctionType
    DR = mybir.MatmulPerfMode.DoubleRow

    nc = bacc.Bacc("TRN2", target_bir_lowering=False, debug=False,
                   num_devices=NCORES)

    dxT = nc.dram_tensor("xT", [P, NSLOT * W], bf, kind="ExternalInput").ap()
    dyT = nc.dram_tensor("yT", [P, NSLOT * W], bf, kind="ExternalInput").ap()
    dxb = nc.dram_tensor("xb2", [P, BLK], bf, kind="ExternalInput").ap()
    dyb = nc.dram_tensor("yb2", [P, BLK], bf, kind="ExternalInput").ap()
    drn = nc.dram_tensor("rn", [P, 2 * CI_N], f32, kind="ExternalInput").ap()
    dcnx = nc.dram_tensor("cnx", [2, NSLOT * W], bf, kind="ExternalInput").ap()
    dcny = nc.dram_tensor("cny", [2, NSLOT * W], bf, kind="ExternalInput").ap()
    dey = nc.dram_tensor("eyew", [P, 2 * 384], f8, kind="ExternalInput").ap()
    dres = nc.dram_tensor("res", [P, 128], f32, kind="ExternalOutput").ap()
    dcols = nc.dram_tensor("cols", [1, 12 * 512], f32, kind="ExternalOutput").ap()

    with tile.TileContext(nc) as tc:
        with tc.tile_pool(name="const", bufs=1) as cp, \
             tc.tile_pool(name="psum", bufs=1, space="PSUM") as pp, \
             tc.tile_pool(name="ab", bufs=3) as abp, \
             tc.tile_pool(name="trd", bufs=2) as trd:

            # ── persistent operands ────────────────────────────────────
            xTt = cp.tile([P, NSLOT * W], bf, tag="xTt")
            yTt = cp.tile([P, NSLOT * W], bf, tag="yTt")
            xb2 = cp.tile([P, BLK], bf, tag="xb2")
            yb2 = cp.tile([P, BLK], bf, tag="yb2")
            rnt = cp.tile([P, 2 * CI_N], f32, tag="rnt")
            cntx = cp.tile([2, NSLOT * W], bf, tag="cntx")
            cnty = cp.tile([2, NSLOT * W], bf, tag="cnty")
            eyew = cp.tile([P, 2, 384], f8, tag="eyew")
            onesf = cp.tile([P, 1], f32, tag="onesf")
            nc.vector.memset(onesf[:], 1.0)
            ones1 = cp.tile([P, 1], f32r, tag="ones1")   # colsum lhsT (f32r)
            nc.vector.tensor_copy(ones1[:], onesf[:])
            res = cp.tile([P, 128], f32, tag="res")
            nc.vector.memset(res[:], 0.0)
            colstage = cp.tile([1, 12 * 512], f32, tag="colstage")

            # ACT sqrt-table preload (avoid a mid-loop ACT_TABLE_LOAD)
            sone = cp.tile([P, 1], f32, tag="sone")
            nc.vector.memset(sone[:], 1.0)
            sdum = cp.tile([P, 1], f32, tag="sdum")
            nc.scalar.activation(sdum[:], sone[:], AF.Sqrt)

            # PE warm-up: dense back-to-back matmuls on constant data so the
            # HAM clock reaches full speed before real work starts
            wur = cp.tile([2, 512], bf, tag="wur")
            nc.vector.memset(wur[:], 0.0)
            ones2b = cp.tile([2, P], bf, tag="ones2b")
            nc.vector.memset(ones2b[:], 1.0)
            wt = pp.tile([P, W], f32, tag="a", bufs=1)
            for _ in range(40):
                nc.tensor.matmul(wt[:, 0:512], ones2b[:], wur[:],
                                 start=True, stop=True)

            # ── input DMAs (small/critical first; windows stream in) ──
            nc.sync.dma_start(rnt[:], drn[:])
            nc.sync.dma_start(cntx[:], dcnx[:])
            nc.sync.dma_start(cnty[:], dcny[:])
            nc.sync.dma_start(eyew[:], dey[:])
            nc.sync.dma_start(xb2[:], dxb[:])
            nc.sync.dma_start(yb2[:], dyb[:])
            w0 = bass.ts(0, W)
            rest = bass.ds(W, (NSLOT - 1) * W)
            nc.sync.dma_start(xTt[:, w0], dxT[:, w0])
            nc.sync.dma_start(yTt[:, w0], dyT[:, w0])
            nc.sync.dma_start(xTt[:, rest], dxT[:, rest])
            nc.sync.dma_start(yTt[:, rest], dyT[:, rest])

            # ── main loop ─────────────────────────────────────────────
            for s in range(NSLOT):
                # slots 0/4 run the same (discarded) colsum matmuls so the
                # PE load stays uniform and the HAM clock does not decay
                cst = pp.tile([1, 4 * 512], f32, tag="cs", bufs=1)
                keep_cs = 1 <= s <= 3
                pend = None
                for ci in range(CI_N):
                    col = s * CI_N + ci
                    psA = pp.tile([P, W], f32, tag="a", bufs=1)
                    psB = pp.tile([P, W], f32, tag="b", bufs=1)
                    for ps_, blk2, full, cnt in ((psA, xb2, xTt, cntx),
                                                 (psB, yb2, yTt, cnty)):
                        for h in range(2):
                            nc.tensor.matmul(
                                ps_[:, bass.ds(h * 512, 512)],
                                blk2[:, bass.ts(ci, P)],
                                full[:, bass.ds(s * W + h * 512, 512)],
                                start=True, stop=False)
                        if s == 0:
                            # += mu^2*I on this chunk's diagonal sub-block
                            qd = ci // 2
                            off = 128 * ((ci + 1) % 2)
                            nc.tensor.matmul(
                                ps_[:, bass.ds(qd * 256, 256)],
                                eyew[:, :, 128:256],
                                eyew[:, :, bass.ds(off, 256)],
                                start=False, stop=False, perf_mode=DR)
                        for h in range(2):
                            nc.tensor.matmul(
                                ps_[:, bass.ds(h * 512, 512)],
                                ones2b[:],
                                cnt[:, bass.ds(s * W + h * 512, 512)],
                                start=False, stop=True)

                    aT = abp.tile([P, W], f32r, tag="a")
                    bT = abp.tile([P, W], f32r, tag="b")
                    nc.scalar.activation(aT[:], psA[:], AF.Sqrt,
                                         bias=rnt[:, ci:ci + 1],
                                         accum_out=res[:, col:col + 1])
                    nc.scalar.activation(bT[:], psB[:], AF.Sqrt,
                                         bias=rnt[:, CI_N + ci:CI_N + ci + 1])
                    nc.vector.tensor_reduce(res[:, 40 + col:41 + col], bT[:],
                                            axis=mybir.AxisListType.X, op=A.add)
                    t0 = trd.tile([P, W], f32, tag="t")
                    nc.vector.scalar_tensor_tensor(
                        t0[:], aT[:], MU, bT[:], op0=A.subtract, op1=A.mult,
                        accum_out=res[:, 80 + col:81 + col])
                    if pend is not None:
                        _emit_cs(nc, cst, ones1, pend, CI_N)
                    pend = (aT, bT, ci)
                _emit_cs(nc, cst, ones1, pend, CI_N)
                if keep_cs:
                    nc.vector.tensor_copy(
                        colstage[0:1, bass.ts(s - 1, 2048)], cst[:])

            nc.sync.dma_start(dres[:], res[:])
            nc.sync.dma_start(dcols[:], colstage[:])

    nc.compile()
    return nc


def _get_program(mm_mode="f32r"):
    if mm_mode not in _programs:
        _programs[mm_mode] = _build()
    return _programs[mm_mode]


def _bf16_terms(v, k=2):
    """Successive bf16 split: v ~= sum of k bf16-representable terms."""
    r = np.asarray(v, np.float64).copy()
    terms = []
    for _ in range(k):
        t = r.astype(BF).astype(np.float64)
        terms.append(t)
        r -= t
    return terms


def _host_quant(x):
    """Per-matrix host-side quantities (fp64): norms and fp8 colnorm terms.

    The device consumes bf16(x); all norms come from those exact values."""
    x64 = np.asarray(x, np.float32).astype(BF).astype(np.float64)
    n_exact = (x64 * x64).sum(1)                       # [N]
    rn = n_exact.astype(np.float32).astype(np.float64)  # shipped fp32 bias
    terms = _bf16_terms(n_exact, 2)
    cn = terms[0] + terms[1]
    return n_exact, rn, terms, cn


def make_in_maps(x, y):
    x = np.ascontiguousarray(np.asarray(x, np.float32))
    y = np.ascontiguousarray(np.asarray(y, np.float32))
    _, rnx, tx, _ = _host_quant(x)
    _, rny, ty, _ = _host_quant(y)
    xT = x.astype(BF).T  # [128, 8192] bf16
    yT = y.astype(BF).T

    # eyew[p, 0, k] = 16*delta(p == k-128); plane 1 zero
    eyew = np.zeros((P, 2, 384), np.float32)
    for p in range(P):
        eyew[p, 0, p + 128] = MU
    eyew8 = eyew.astype(F8).reshape(P, 2 * 384)

    in_maps = []
    for c in range(NCORES):
        wins = [(c + s) % NCORES for s in range(NSLOT)]
        colsel = np.concatenate([np.arange(w * W, (w + 1) * W) for w in wins])
        rn = np.empty((P, 2 * CI_N), np.float32)
        for ci in range(CI_N):
            base = c * BLK + ci * P
            rn[:, ci] = rnx[base:base + P]
            rn[:, CI_N + ci] = rny[base:base + P]

        def cn_pack(terms):
            # [2, NSLOT*W]: row 0 = hi, row 1 = lo (bf16)
            out = np.zeros((2, NSLOT * W), np.float32)
            out[0] = terms[0][colsel]
            out[1] = terms[1][colsel]
            return out.astype(BF)

        in_maps.append({
            "xT": np.ascontiguousarray(xT[:, colsel]),
            "yT": np.ascontiguousarray(yT[:, colsel]),
            "xb2": np.ascontiguousarray(
                (-2.0 * xT[:, c * BLK:(c + 1) * BLK].astype(np.float32))
                .astype(BF)),
            "yb2": np.ascontiguousarray(
                (-2.0 * yT[:, c * BLK:(c + 1) * BLK].astype(np.float32))
                .astype(BF)),
            "rn": rn,
            "cnx": cn_pack(tx),
            "cny": cn_pack(ty),
            "eyew": eyew8,
        })
    return in_maps


def finalize(outs, x, y):
    """outs: list of 8 dicts with 'res' [128,128] and 'cols' [4, 3072].

    res cols: rs_a 0:40 | rs_b 40:80 | pab 80:120, col = s*8+ci, value at
    partition p belongs to row c*1024+ci*128+p.
    cols rows: 0/1 = a-tile column sums (halves 0/1), 2/3 = same for b;
    slot s occupies cols (s-1)*512 : s*512.
    """
    n = float(N)
    nx, rnx, _, cnx = _host_quant(x)
    ny, rny, _, cny = _host_quant(y)
    x64 = np.asarray(x, np.float32).astype(BF).astype(np.float64)
    y64 = np.asarray(y, np.float32).astype(BF).astype(np.float64)

    res = [np.asarray(o["res"], np.float64) for o in outs]
    cols = [np.asarray(o["cols"], np.float64) for o in outs]

    rs_a = np.empty(N)
    rs_b = np.empty(N)
    pab = 0.0
    wslot = np.array([1.0, 2.0, 2.0, 2.0, 1.0])
    for c in range(NCORES):
        r = res[c]
        st0 = r[:, 0:40].reshape(P, NSLOT, CI_N)    # [p, s, ci]
        st1 = r[:, 40:80].reshape(P, NSLOT, CI_N)
        st2 = r[:, 80:120].reshape(P, NSLOT, CI_N)
        own_a = st0.sum(axis=1)                     # [p, ci]
        own_b = st1.sum(axis=1)
        # mirrored contributions: window (c+d)%8, d=5,6,7 -> core m slot 8-d
        mir_a = np.zeros(BLK)
        mir_b = np.zeros(BLK)
        for d in (5, 6, 7):
            m = (c + d) % NCORES
            sp = 8 - d
            base = (sp - 1) * 2048
            cv = cols[m][0]
            mir_a += np.concatenate([cv[base:base + 512],
                                     cv[base + 512:base + 1024]])
            mir_b += np.concatenate([cv[base + 1024:base + 1536],
                                     cv[base + 1536:base + 2048]])
        blk_a = own_a.T.ravel() + mir_a             # [1024], ci-major
        blk_b = own_b.T.ravel() + mir_b
        rs_a[c * BLK:(c + 1) * BLK] = blk_a
        rs_b[c * BLK:(c + 1) * BLK] = blk_b
        pab += (st2.sum(axis=(0, 2)) * wslot).sum()

    # closed-form sums of device sq over all ij (fp64, host-exact)
    sum_sq_a = n * rnx.sum() + n * cnx.sum() - 2.0 * (x64.sum(0) @ x64.sum(0))
    sum_sq_b = n * rny.sum() + n * cny.sum() - 2.0 * (y64.sum(0) @ y64.sum(0))
    diag_sq_a = (rnx + cnx - 2.0 * nx).sum()
    diag_sq_b = (rny + cny - 2.0 * ny).sum()

    sa = rs_a - MU          # true rowsums (device diag sqrt(256+eps) ~ 16)
    sb = rs_b - MU
    Sq_a_off = sum_sq_a - diag_sq_a
    Sq_b_off = sum_sq_b - diag_sq_b
    sat = sa - n * MU
    sbt = sb - n * MU
    Ua, Ub = sat.sum(), sbt.sum()
    # device pab = weighted sum of (a-mu)*b; forced diag contributes
    # (16-16)*16 = 0, matching the true (0-mu)*0 = 0.
    # Sab = sum over all ij of (a_true - mu)(b_true - mu)
    Sab = pab - MU * (sa.sum() - MU * n * n)
    Saa = Sq_a_off - 2.0 * MU * sa.sum() + MU * MU * n * n
    Sbb = Sq_b_off - 2.0 * MU * sb.sum() + MU * MU * n * n

    sumAB = Sab - 2.0 * np.dot(sat, sbt) / n + Ua * Ub / n ** 2
    sumAA = Saa - 2.0 * np.dot(sat, sat) / n + Ua * Ua / n ** 2
    sumBB = Sbb - 2.0 * np.dot(sbt, sbt) / n + Ub * Ub / n ** 2

    inv_n2 = 1.0 / (n * n)
    dcor = (-np.sqrt(sumAB * inv_n2)
            / np.sqrt(np.sqrt(sumAA * inv_n2) * np.sqrt(sumBB * inv_n2)))
    return np.asarray(dcor, dtype=np.float32)


def run(x, y, mm_mode=None, trace=False, tmpdir=None):
    nc = _get_program()
    in_maps = make_in_maps(x, y)
    res = run_bass_kernel_spmd(nc, in_maps, core_ids=list(range(NCORES)),
                               trace=trace, tmpdir=tmpdir)
    return finalize(res.results, x, y), res


def kernel(x, y):
    val, _ = run(x, y)
    return val


# revision 34
# speedup vs baseline: 1.0961x; 1.0961x over previous
"""Distance-correlation (DcorLoss) kernel for 8 trn2 NeuronCores.

Math: for x, y [n=8192, d=128]:
  a = pairwise_dist(x), b = pairwise_dist(y)   (n x n, symmetric, zero diag)
  A = double_center(a), B = double_center(b)
  dcor = -sqrt(sum(A*B)) / sqrt(sqrt(sum(A*A)) * sqrt(sum(B*B)))

Identities (never materialize A/B):
  sum(HaH o HbH) = sum(at o bt) - 2/n * dot(rs_at, rs_bt) + sum(at)*sum(bt)/n^2
with at = a - mu. sum(a-mu)^2 via the closed form for sum a^2 = sum sq.
Only sum (a-mu)*b and the row sums of a/b need streaming the matrices.

Symmetric block coverage: core c owns row block c (1024 rows). Each unordered
block pair {r, j} is computed once: core c runs 5 column-window "slots"
s=0..4 over windows (c+s) mod 8. Slot 0 = diagonal block, slots 1-3 pairs
counted twice on host, slot 4 pair computed by both ends (counted once each).
Row sums for the mirrored (uncomputed) windows of block c come from COLUMN
sums of slots 1-3 tiles of cores (c+5..c+7) mod 8, computed on-device with
ones^T matmuls accumulated in PSUM across the 8 row chunks of a slot.

Per (128-row x 1024-col) tile pair, the device computes:
  PE:   psum = -2*x_blk^T x  (bf16 K=128) + column norms (bf16 hi/lo K=2)
        + mu^2*I on the slot-0 diagonal sub-block (fp8 DoubleRow identity)
  ACT:  t = sqrt(psum + n_i)  [fp32 row-norm bias], float32r out; the a-side
        carries accum_out -> row sums (consistent: all consumers see the
        same ~fp32 values, which keeps the closed-form sum-of-squares and
        the streamed sums in agreement -- bf16 outputs here cost 2e-2 rel)
  DVE:  row-sum reduce of t_b, (t_a - mu) * t_b accum (stt)
  PE:   ones^T t_a / t_b column sums (slots 1-3, f32r, dst partition 0),
        deferred one iteration so they never block the next psum fill
Cross-core combining is fp64 on host (partials are tiny).
"""

import os

import numpy as np
import ml_dtypes

import concourse.bass as bass
import concourse.tile as tile
from concourse import bacc, mybir
from concourse.bass_utils import run_bass_kernel_spmd

P = 128            # partitions / d
N = 8192           # points
NCORES = 8
BLK = N // NCORES  # 1024 rows per core
CI_N = BLK // P    # 8 row chunks per core
W = 1024           # column window
NSLOT = 5          # symmetric coverage slots
MU = 16.0
F8 = ml_dtypes.float8_e4m3
BF = ml_dtypes.bfloat16

_programs = {}


def _emit_cs(nc, cst, ones1, pend, ci_n):
    aT, bT, ci = pend
    for r, (ssrc, h) in enumerate(((aT, 0), (aT, 1), (bT, 0), (bT, 1))):
        nc.tensor.matmul(
            cst[0:1, r * 512:(r + 1) * 512], ones1[:],
            ssrc[:, bass.ts(h, 512)],
            start=(ci == 0), stop=(ci == ci_n - 1),
            skip_group_check=True, tile_position=(0, 0))


def _build():
    dt = mybir.dt
    f32 = dt.float32
    f32r = dt.float32r
    f8 = dt.float8e4
    bf = dt.bfloat16
    A = mybir.AluOpType
    AF = mybir.ActivationFunctionType
    DR = mybir.MatmulPerfMode.DoubleRow

    nc = bacc.Bacc("TRN2", target_bir_lowering=False, debug=False,
                   num_devices=NCORES)

    dxT = nc.dram_tensor("xT", [P, NSLOT * W], bf, kind="ExternalInput").ap()
    dyT = nc.dram_tensor("yT", [P, NSLOT * W], bf, kind="ExternalInput").ap()
    dxb = nc.dram_tensor("xb2", [P, BLK], bf, kind="ExternalInput").ap()
    dyb = nc.dram_tensor("yb2", [P, BLK], bf, kind="ExternalInput").ap()
    drn = nc.dram_tensor("rn", [P, 2 * CI_N], f32, kind="ExternalInput").ap()
    dcnx = nc.dram_tensor("cnx", [2, NSLOT * W], bf, kind="ExternalInput").ap()
    dcny = nc.dram_tensor("cny", [2, NSLOT * W], bf, kind="ExternalInput").ap()
    dey = nc.dram_tensor("eyew", [P, 2 * 384], f8, kind="ExternalInput").ap()
    dres = nc.dram_tensor("res", [P, 128], f32, kind="ExternalOutput").ap()
    dcols = nc.dram_tensor("cols", [1, 12 * 512], f32, kind="ExternalOutput").ap()

    with tile.TileContext(nc) as tc:
        with tc.tile_pool(name="const", bufs=1) as cp, \
             tc.tile_pool(name="psum", bufs=1, space="PSUM") as pp, \
             tc.tile_pool(name="ab", bufs=3) as abp, \
             tc.tile_pool(name="trd", bufs=2) as trd:

            # ── persistent operands ────────────────────────────────────
            xTt = cp.tile([P, NSLOT * W], bf, tag="xTt")
            yTt = cp.tile([P, NSLOT * W], bf, tag="yTt")
            xb2 = cp.tile([P, BLK], bf, tag="xb2")
            yb2 = cp.tile([P, BLK], bf, tag="yb2")
            rnt = cp.tile([P, 2 * CI_N], f32, tag="rnt")
            cntx = cp.tile([2, NSLOT * W], bf, tag="cntx")
            cnty = cp.tile([2, NSLOT * W], bf, tag="cnty")
            eyew = cp.tile([P, 2, 384], f8, tag="eyew")
            onesf = cp.tile([P, 1], f32, tag="onesf")
            nc.vector.memset(onesf[:], 1.0)
            ones1 = cp.tile([P, 1], f32r, tag="ones1")   # colsum lhsT (f32r)
            nc.vector.tensor_copy(ones1[:], onesf[:])
            res = cp.tile([P, 128], f32, tag="res")
            nc.vector.memset(res[:], 0.0)
            colstage = cp.tile([1, 12 * 512], f32, tag="colstage")

            # ACT sqrt-table preload (avoid a mid-loop ACT_TABLE_LOAD)
            sone = cp.tile([P, 1], f32, tag="sone")
            nc.vector.memset(sone[:], 1.0)
            sdum = cp.tile([P, 1], f32, tag="sdum")
            nc.scalar.activation(sdum[:], sone[:], AF.Sqrt)

            # PE warm-up: dense back-to-back matmuls on constant data so the
            # HAM clock reaches full speed before real work starts
            wur = cp.tile([2, 512], bf, tag="wur")
            nc.vector.memset(wur[:], 0.0)
            ones2b = cp.tile([2, P], bf, tag="ones2b")
            nc.vector.memset(ones2b[:], 1.0)
            wt = pp.tile([P, W], f32, tag="a", bufs=1)
            for _ in range(12):
                nc.tensor.matmul(wt[:, 0:512], ones2b[:], wur[:],
                                 start=True, stop=True)

            # ── input DMAs (small/critical first; windows stream in) ──
            nc.sync.dma_start(rnt[:], drn[:])
            nc.sync.dma_start(cntx[:], dcnx[:])
            nc.sync.dma_start(cnty[:], dcny[:])
            nc.sync.dma_start(eyew[:], dey[:])
            nc.sync.dma_start(xb2[:], dxb[:])
            nc.sync.dma_start(yb2[:], dyb[:])
            w0 = bass.ts(0, W)
            rest = bass.ds(W, (NSLOT - 1) * W)
            nc.sync.dma_start(xTt[:, w0], dxT[:, w0])
            nc.sync.dma_start(yTt[:, w0], dyT[:, w0])
            nc.sync.dma_start(xTt[:, rest], dxT[:, rest])
            nc.sync.dma_start(yTt[:, rest], dyT[:, rest])

            # ── main loop ─────────────────────────────────────────────
            for s in range(NSLOT):
                # slots 0/4 run the same (discarded) colsum matmuls so the
                # PE load stays uniform and the HAM clock does not decay
                cst = pp.tile([1, 4 * 512], f32, tag="cs", bufs=1)
                keep_cs = 1 <= s <= 3
                pend = None
                for ci in range(CI_N):
                    col = s * CI_N + ci
                    psA = pp.tile([P, W], f32, tag="a", bufs=1)
                    psB = pp.tile([P, W], f32, tag="b", bufs=1)
                    for ps_, blk2, full, cnt in ((psA, xb2, xTt, cntx),
                                                 (psB, yb2, yTt, cnty)):
                        for h in range(2):
                            nc.tensor.matmul(
                                ps_[:, bass.ds(h * 512, 512)],
                                blk2[:, bass.ts(ci, P)],
                                full[:, bass.ds(s * W + h * 512, 512)],
                                start=True, stop=False)
                        if s == 0:
                            # += mu^2*I on this chunk's diagonal sub-block
                            qd = ci // 2
                            off = 128 * ((ci + 1) % 2)
                            nc.tensor.matmul(
                                ps_[:, bass.ds(qd * 256, 256)],
                                eyew[:, :, 128:256],
                                eyew[:, :, bass.ds(off, 256)],
                                start=False, stop=False, perf_mode=DR)
                        for h in range(2):
                            nc.tensor.matmul(
                                ps_[:, bass.ds(h * 512, 512)],
                                ones2b[:],
                                cnt[:, bass.ds(s * W + h * 512, 512)],
                                start=False, stop=True)

                    aT = abp.tile([P, W], f32r, tag="a")
                    bT = abp.tile([P, W], f32r, tag="b")
                    nc.scalar.activation(aT[:], psA[:], AF.Sqrt,
                                         bias=rnt[:, ci:ci + 1],
                                         accum_out=res[:, col:col + 1])
                    nc.scalar.activation(bT[:], psB[:], AF.Sqrt,
                                         bias=rnt[:, CI_N + ci:CI_N + ci + 1])
                    nc.vector.tensor_reduce(res[:, 40 + col:41 + col], bT[:],
                                            axis=mybir.AxisListType.X, op=A.add)
                    t0 = trd.tile([P, W], f32, tag="t")
                    nc.vector.scalar_tensor_tensor(
                        t0[:], aT[:], MU, bT[:], op0=A.subtract, op1=A.mult,
                        accum_out=res[:, 80 + col:81 + col])
                    if pend is not None:
                        _emit_cs(nc, cst, ones1, pend, CI_N)
                    pend = (aT, bT, ci)
                _emit_cs(nc, cst, ones1, pend, CI_N)
                if keep_cs:
                    nc.vector.tensor_copy(
                        colstage[0:1, bass.ts(s - 1, 2048)], cst[:])

            nc.sync.dma_start(dres[:], res[:])
            nc.sync.dma_start(dcols[:], colstage[:])

    nc.compile()
    return nc


def _get_program(mm_mode="f32r"):
    if mm_mode not in _programs:
        _programs[mm_mode] = _build()
    return _programs[mm_mode]


def _bf16_terms(v, k=2):
    """Successive bf16 split: v ~= sum of k bf16-representable terms."""
    r = np.asarray(v, np.float64).copy()
    terms = []
    for _ in range(k):
        t = r.astype(BF).astype(np.float64)
        terms.append(t)
        r -= t
    return terms


def _host_quant(x):
    """Per-matrix host-side quantities (fp64): norms and fp8 colnorm terms.

    The device consumes bf16(x); all norms come from those exact values."""
    x64 = np.asarray(x, np.float32).astype(BF).astype(np.float64)
    n_exact = (x64 * x64).sum(1)                       # [N]
    rn = n_exact.astype(np.float32).astype(np.float64)  # shipped fp32 bias
    terms = _bf16_terms(n_exact, 2)
    cn = terms[0] + terms[1]
    return n_exact, rn, terms, cn


def make_in_maps(x, y):
    x = np.ascontiguousarray(np.asarray(x, np.float32))
    y = np.ascontiguousarray(np.asarray(y, np.float32))
    _, rnx, tx, _ = _host_quant(x)
    _, rny, ty, _ = _host_quant(y)
    xT = x.astype(BF).T  # [128, 8192] bf16
    yT = y.astype(BF).T

    # eyew[p, 0, k] = 16*delta(p == k-128); plane 1 zero
    eyew = np.zeros((P, 2, 384), np.float32)
    for p in range(P):
        eyew[p, 0, p + 128] = MU
    eyew8 = eyew.astype(F8).reshape(P, 2 * 384)

    in_maps = []
    for c in range(NCORES):
        wins = [(c + s) % NCORES for s in range(NSLOT)]
        colsel = np.concatenate([np.arange(w * W, (w + 1) * W) for w in wins])
        rn = np.empty((P, 2 * CI_N), np.float32)
        for ci in range(CI_N):
            base = c * BLK + ci * P
            rn[:, ci] = rnx[base:base + P]
            rn[:, CI_N + ci] = rny[base:base + P]

        def cn_pack(terms):
            # [2, NSLOT*W]: row 0 = hi, row 1 = lo (bf16)
            out = np.zeros((2, NSLOT * W), np.float32)
            out[0] = terms[0][colsel]
            out[1] = terms[1][colsel]
            return out.astype(BF)

        in_maps.append({
            "xT": np.ascontiguousarray(xT[:, colsel]),
            "yT": np.ascontiguousarray(yT[:, colsel]),
            "xb2": np.ascontiguousarray(
                (-2.0 * xT[:, c * BLK:(c + 1) * BLK].astype(np.float32))
                .astype(BF)),
            "yb2": np.ascontiguousarray(
                (-2.0 * yT[:, c * BLK:(c + 1) * BLK].astype(np.float32))
                .astype(BF)),
            "rn": rn,
            "cnx": cn_pack(tx),
            "cny": cn_pack(ty),
            "eyew": eyew8,
        })
    return in_maps


def finalize(outs, x, y):
    """outs: list of 8 dicts with 'res' [128,128] and 'cols' [4, 3072].

    res cols: rs_a 0:40 | rs_b 40:80 | pab 80:120, col = s*8+ci, value at
    partition p belongs to row c*1024+ci*128+p.
    cols rows: 0/1 = a-tile column sums (halves 0/1), 2/3 = same for b;
    slot s occupies cols (s-1)*512 : s*512.
    """
    n = float(N)
    nx, rnx, _, cnx = _host_quant(x)
    ny, rny, _, cny = _host_quant(y)
    x64 = np.asarray(x, np.float32).astype(BF).astype(np.float64)
    y64 = np.asarray(y, np.float32).astype(BF).astype(np.float64)

    res = [np.asarray(o["res"], np.float64) for o in outs]
    cols = [np.asarray(o["cols"], np.float64) for o in outs]

    rs_a = np.empty(N)
    rs_b = np.empty(N)
    pab = 0.0
    wslot = np.array([1.0, 2.0, 2.0, 2.0, 1.0])
    for c in range(NCORES):
        r = res[c]
        st0 = r[:, 0:40].reshape(P, NSLOT, CI_N)    # [p, s, ci]
        st1 = r[:, 40:80].reshape(P, NSLOT, CI_N)
        st2 = r[:, 80:120].reshape(P, NSLOT, CI_N)
        own_a = st0.sum(axis=1)                     # [p, ci]
        own_b = st1.sum(axis=1)
        # mirrored contributions: window (c+d)%8, d=5,6,7 -> core m slot 8-d
        mir_a = np.zeros(BLK)
        mir_b = np.zeros(BLK)
        for d in (5, 6, 7):
            m = (c + d) % NCORES
            sp = 8 - d
            base = (sp - 1) * 2048
            cv = cols[m][0]
            mir_a += np.concatenate([cv[base:base + 512],
                                     cv[base + 512:base + 1024]])
            mir_b += np.concatenate([cv[base + 1024:base + 1536],
                                     cv[base + 1536:base + 2048]])
        blk_a = own_a.T.ravel() + mir_a             # [1024], ci-major
        blk_b = own_b.T.ravel() + mir_b
        rs_a[c * BLK:(c + 1) * BLK] = blk_a
        rs_b[c * BLK:(c + 1) * BLK] = blk_b
        pab += (st2.sum(axis=(0, 2)) * wslot).sum()

    # closed-form sums of device sq over all ij (fp64, host-exact)
    sum_sq_a = n * rnx.sum() + n * cnx.sum() - 2.0 * (x64.sum(0) @ x64.sum(0))
    sum_sq_b = n * rny.sum() + n * cny.sum() - 2.0 * (y64.sum(0) @ y64.sum(0))
    diag_sq_a = (rnx + cnx - 2.0 * nx).sum()
    diag_sq_b = (rny + cny - 2.0 * ny).sum()

    sa = rs_a - MU          # true rowsums (device diag sqrt(256+eps) ~ 16)
    sb = rs_b - MU
    Sq_a_off = sum_sq_a - diag_sq_a
    Sq_b_off = sum_sq_b - diag_sq_b
    sat = sa - n * MU
    sbt = sb - n * MU
    Ua, Ub = sat.sum(), sbt.sum()
    # device pab = weighted sum of (a-mu)*b; forced diag contributes
    # (16-16)*16 = 0, matching the true (0-mu)*0 = 0.
    # Sab = sum over all ij of (a_true - mu)(b_true - mu)
    Sab = pab - MU * (sa.sum() - MU * n * n)
    Saa = Sq_a_off - 2.0 * MU * sa.sum() + MU * MU * n * n
    Sbb = Sq_b_off - 2.0 * MU * sb.sum() + MU * MU * n * n

    sumAB = Sab - 2.0 * np.dot(sat, sbt) / n + Ua * Ub / n ** 2
    sumAA = Saa - 2.0 * np.dot(sat, sat) / n + Ua * Ua / n ** 2
    sumBB = Sbb - 2.0 * np.dot(sbt, sbt) / n + Ub * Ub / n ** 2

    inv_n2 = 1.0 / (n * n)
    dcor = (-np.sqrt(sumAB * inv_n2)
            / np.sqrt(np.sqrt(sumAA * inv_n2) * np.sqrt(sumBB * inv_n2)))
    return np.asarray(dcor, dtype=np.float32)


def run(x, y, mm_mode=None, trace=False, tmpdir=None):
    nc = _get_program()
    in_maps = make_in_maps(x, y)
    res = run_bass_kernel_spmd(nc, in_maps, core_ids=list(range(NCORES)),
                               trace=trace, tmpdir=tmpdir)
    return finalize(res.results, x, y), res


def kernel(x, y):
    val, _ = run(x, y)
    return val
